# revision 5
# baseline (speedup 1.0000x reference)
import sys

sys.path.insert(0, "/opt/trn_rl_repo")

from contextlib import ExitStack

import numpy as np

import concourse.bass as bass
from concourse import bacc
import concourse.tile as tile
from concourse import mybir
from concourse.bass import ds
from concourse.bass_utils import run_bass_kernel_spmd

# Problem constants (nn_AudioEncoder: 4-layer bidirectional LSTM w/ projection)
B, T, EMBED = 16, 2048, 1024
H = 256      # cell size
P = 128      # proj size
L = 4        # layers
NCORES = 8
B2 = B // NCORES   # sequences per core
NG = 8             # gate chunks of 128 (4H = 1024)
# chunk reorder: pytorch gate order [i,f,g,o] -> [i,f,o,g] so sigmoid gates are contiguous
PERM = [0, 1, 2, 3, 6, 7, 4, 5]

BLK = 32           # recurrent steps per block
UNROLL = 2         # blocks per For_i iteration
NTT = 256          # t-steps per GEMM token tile (512 tokens)

F32 = mybir.dt.float32
AF = mybir.ActivationFunctionType


def build_nc(Tloc=T):
    nt_tiles = Tloc // NTT
    nc = bacc.Bacc()
    # ---- DRAM I/O (per core) ----
    xT = nc.dram_tensor("xT", [NG, 128, Tloc, B2], F32, kind="ExternalInput")
    wih0 = nc.dram_tensor("wih0", [2, 8, 128, 1024], F32, kind="ExternalInput")
    wih123 = nc.dram_tensor("wih123", [3, 2, 2, 128, 1024], F32, kind="ExternalInput")
    whh = nc.dram_tensor("whh", [L, 2, 128, 1024], F32, kind="ExternalInput")
    whr = nc.dram_tensor("whr", [L, 2, 2, 128, 128], F32, kind="ExternalInput")
    bias = nc.dram_tensor("bias", [L, 2, 128, 8], F32, kind="ExternalInput")
    outf = nc.dram_tensor("outf", [128, Tloc, B2], F32, kind="ExternalOutput")
    outb = nc.dram_tensor("outb", [128, Tloc, B2], F32, kind="ExternalOutput")

    with tile.TileContext(nc) as tc:
        with (
            tc.tile_pool(name="dram", bufs=1, space="DRAM") as dpool,
            tc.tile_pool(name="hbufs", bufs=1) as hpool,
        ):
            # xp scratch in DRAM: per dir [128, NG, T, B2]
            xp_dram = [dpool.tile([128, NG, Tloc, B2], F32, tag=f"xp{d}", name=f"xp{d}") for d in range(2)]
            # layer-output h buffers (SBUF resident), 2 generations x 2 dirs
            hout = [
                [hpool.tile([128, Tloc, B2], F32, tag=f"hout{g}{d}", name=f"hout{g}{d}") for d in range(2)]
                for g in range(2)
            ]

            for l in range(L):
                gen, pgen = l % 2, (l - 1) % 2
                kchunks = 8 if l == 0 else 2
                with ExitStack() as layer_ctx:
                    wl = layer_ctx.enter_context(tc.tile_pool(name=f"wl{l}", bufs=1))
                    # ---- load this layer's weights ----
                    wih_sb = wl.tile([128, 2, kchunks, 1024], F32, tag="wih")
                    for d in range(2):
                        for k in range(kchunks):
                            src = wih0[d, k] if l == 0 else wih123[l - 1, d, k]
                            nc.gpsimd.dma_start(wih_sb[:, d, k, :], src)
                    whh_sb = wl.tile([128, 2, 1024], F32, tag="whh")
                    whr_sb = wl.tile([128, 2, 2, 128], F32, tag="whr")
                    bias_sb = wl.tile([128, 2, 8], F32, tag="bias")
                    for d in range(2):
                        nc.gpsimd.dma_start(whh_sb[:, d, :], whh[l, d])
                        for kc in range(2):
                            nc.gpsimd.dma_start(whr_sb[:, d, kc, :], whr[l, d, kc])
                        nc.gpsimd.dma_start(bias_sb[:, d, :], bias[l, d])

                    # ---- GEMM phase: xp[d] = W_ih @ input + b  (all t) ----
                    with (
                        tc.tile_pool(name=f"gemm{l}", bufs=2) as gp,
                        tc.tile_pool(name=f"gstage{l}", bufs=4) as gsp,
                        tc.tile_pool(name=f"gpsum{l}", bufs=4, space="PSUM") as gps,
                    ):
                        for n in range(nt_tiles):
                            m = nt_tiles - 1 - n  # mirrored tile (reversed-time reads)
                            if l == 0:
                                xtiles = []
                                for k in range(kchunks):
                                    xt = gp.tile([128, NTT, B2], F32, tag=f"xt{k}")
                                    nc.gpsimd.dma_start(
                                        xt[:], xT[k, :, n * NTT:(n + 1) * NTT, :]
                                    )
                                    xtiles.append(xt)
                                rhs_f = xtiles                           # fwd tile n
                                rhs_b = [x[:, ::-1, :] for x in xtiles]  # bwd tile m
                            else:
                                hf, hb = hout[pgen][0], hout[pgen][1]
                                rhs_f = [
                                    hf[:, n * NTT:(n + 1) * NTT, :],
                                    hb[:, m * NTT:(m + 1) * NTT, :][:, ::-1, :],
                                ]
                                rhs_b = [
                                    hf[:, n * NTT:(n + 1) * NTT, :][:, ::-1, :],
                                    hb[:, m * NTT:(m + 1) * NTT, :],
                                ]
                            for d, rhs, tidx in ((0, rhs_f, n), (1, rhs_b, m)):
                                for j in range(NG):
                                    ps = gps.tile([128, NTT, B2], F32, tag="gemmps")
                                    for k in range(kchunks):
                                        nc.tensor.matmul(
                                            ps[:],
                                            wih_sb[:, d, k, j * 128:(j + 1) * 128],
                                            rhs[k],
                                            start=(k == 0),
                                            stop=(k == kchunks - 1),
                                        )
                                    st = gsp.tile([128, NTT, B2], F32, tag="stage")
                                    nc.scalar.activation(
                                        st[:], ps[:], AF.Identity,
                                        bias=bias_sb[:, d, j:j + 1],
                                    )
                                    nc.sync.dma_start(
                                        xp_dram[d][:, j, tidx * NTT:(tidx + 1) * NTT, :],
                                        st[:],
                                    )

                    # ---- recurrent loop phase ----
                    with (
                        tc.tile_pool(name=f"loop{l}", bufs=1) as lp,
                        tc.tile_pool(name=f"tmp{l}", bufs=2) as tp,
                        tc.tile_pool(name=f"lpsum{l}", bufs=1, space="PSUM") as lps,
                    ):
                        c_st = [lp.tile([128, 2, B2], F32, tag=f"c{d}", name=f"c{d}") for d in range(2)]
                        xpb = [
                            [lp.tile([128, NG, BLK, B2], F32, tag=f"xpb{d}{p}", name=f"xpb{d}{p}")
                             for p in range(UNROLL)]
                            for d in range(2)
                        ]
                        hst = [
                            [lp.tile([128, BLK, B2], F32, tag=f"hst{d}{p}", name=f"hst{d}{p}")
                             for p in range(UNROLL)]
                            for d in range(2)
                        ]
                        psg = [lps.tile([128, NG, B2], F32, tag=f"psg{d}", name=f"psg{d}") for d in range(2)]
                        psh = [lps.tile([128, B2], F32, tag=f"psh{d}", name=f"psh{d}") for d in range(2)]

                        for d in range(2):
                            nc.vector.memset(c_st[d][:], 0.0)
                            nc.vector.memset(hst[d][UNROLL - 1][:, BLK - 1, :], 0.0)

                        with tc.For_i(0, Tloc, UNROLL * BLK) as i0:
                            for pp in range(UNROLL):
                                off = i0 + pp * BLK
                                for d in range(2):
                                    nc.sync.dma_start(
                                        xpb[d][pp][:], xp_dram[d][:, :, ds(off, BLK), :]
                                    )
                                for k in range(BLK):
                                    for d in range(2):
                                        h_prev = (
                                            hst[d][pp][:, k - 1, :] if k > 0
                                            else hst[d][pp - 1][:, BLK - 1, :]
                                        )
                                        gv = psg[d]
                                        for j in range(NG):
                                            nc.tensor.matmul(
                                                gv[:, j, :],
                                                whh_sb[:, d, j * 128:(j + 1) * 128],
                                                h_prev,
                                                start=True, stop=True,
                                            )
                                        gpre = tp.tile([128, NG, B2], F32, tag=f"gpre{d}")
                                        nc.vector.tensor_add(
                                            gpre[:], gv[:], xpb[d][pp][:, :, k, :]
                                        )
                                        sg = tp.tile([128, 6, B2], F32, tag=f"sg{d}")
                                        nc.scalar.activation(sg[:], gpre[:, 0:6, :], AF.Sigmoid)
                                        gt = tp.tile([128, 2, B2], F32, tag=f"gt{d}")
                                        nc.scalar.activation(gt[:], gpre[:, 6:8, :], AF.Tanh)
                                        ig = tp.tile([128, 2, B2], F32, tag=f"ig{d}")
                                        nc.vector.tensor_mul(ig[:], sg[:, 0:2, :], gt[:])
                                        fc = tp.tile([128, 2, B2], F32, tag=f"fc{d}")
                                        nc.vector.tensor_mul(fc[:], sg[:, 2:4, :], c_st[d][:])
                                        nc.vector.tensor_add(c_st[d][:], ig[:], fc[:])
                                        tch = tp.tile([128, 2, B2], F32, tag=f"tch{d}")
                                        nc.scalar.activation(tch[:], c_st[d][:], AF.Tanh)
                                        s_t = tp.tile([128, 2, B2], F32, tag=f"s{d}")
                                        nc.vector.tensor_mul(s_t[:], sg[:, 4:6, :], tch[:])
                                        nc.tensor.matmul(
                                            psh[d][:], whr_sb[:, d, 0, :], s_t[:, 0, :],
                                            start=True, stop=False,
                                        )
                                        nc.tensor.matmul(
                                            psh[d][:], whr_sb[:, d, 1, :], s_t[:, 1, :],
                                            start=False, stop=True,
                                        )
                                        nc.scalar.copy(hst[d][pp][:, k, :], psh[d][:])
                                # flush h block
                                for d in range(2):
                                    dst = (
                                        hout[gen][d][:, ds(off, BLK), :] if l < L - 1
                                        else (outf if d == 0 else outb)[:, ds(off, BLK), :]
                                    )
                                    nc.sync.dma_start(dst, hst[d][pp][:])
    nc.compile()
    return nc


def _prep_inputs(x, params, Tloc=T):
    """Host-side: shard + layout. Returns list of per-core input dicts."""
    perm = np.array(PERM)
    row_perm = (perm[:, None] * 128 + np.arange(128)[None, :]).reshape(-1)
    wih0_a = np.zeros((2, 8, 128, 1024), np.float32)
    wih123_a = np.zeros((3, 2, 2, 128, 1024), np.float32)
    whh_a = np.zeros((L, 2, 128, 1024), np.float32)
    whr_a = np.zeros((L, 2, 2, 128, 128), np.float32)
    bias_a = np.zeros((L, 2, 128, 8), np.float32)
    for l in range(L):
        for d in range(2):
            W_ih, W_hh, b_ih, b_hh, W_hr = [np.asarray(a, np.float32) for a in params[l][d]]
            W_ih = W_ih[row_perm]       # [1024, in_dim]
            W_hh = W_hh[row_perm]       # [1024, 128]
            bb = (b_ih + b_hh)[row_perm]
            if l == 0:
                wih0_a[d] = np.ascontiguousarray(W_ih.T.reshape(8, 128, 1024))
            else:
                wih123_a[l - 1, d] = np.ascontiguousarray(W_ih.T.reshape(2, 128, 1024))
            whh_a[l, d] = W_hh.T                       # [128, 1024]
            whr_a[l, d] = W_hr.T.reshape(2, 128, 128)  # [kc, 128(h), 128(P)]
            bias_a[l, d] = bb.reshape(8, 128).T
    ins = []
    for c in range(NCORES):
        xc = np.asarray(x[c * B2:(c + 1) * B2, :Tloc], np.float32)  # [B2, T, E]
        xTc = np.ascontiguousarray(xc.transpose(2, 1, 0)).reshape(8, 128, Tloc, B2)
        ins.append(dict(xT=xTc, wih0=wih0_a, wih123=wih123_a, whh=whh_a,
                        whr=whr_a, bias=bias_a))
    return ins


_NC_CACHE = {}


def kernel(x, params):
    x = np.asarray(x)
    Tloc = x.shape[1]
    if Tloc not in _NC_CACHE:
        _NC_CACHE[Tloc] = build_nc(Tloc)
    nc = _NC_CACHE[Tloc]
    ins = _prep_inputs(x, params, Tloc)
    res = run_bass_kernel_spmd(nc, ins, list(range(NCORES))).results
    out = np.zeros((B, Tloc, 2 * P), np.float32)
    for c in range(NCORES):
        hf = res[c]["outf"]  # [128, T, B2] natural t
        hb = res[c]["outb"]  # [128, T, B2] tau = T-1-t
        out[c * B2:(c + 1) * B2, :, :P] = hf.transpose(2, 1, 0)
        out[c * B2:(c + 1) * B2, :, P:] = hb[:, ::-1, :].transpose(2, 1, 0)
    return out


# revision 7
# speedup vs baseline: 1.1476x; 1.1476x over previous
import sys

sys.path.insert(0, "/opt/trn_rl_repo")

from contextlib import ExitStack

import numpy as np

import concourse.bass as bass
from concourse import bacc
import concourse.tile as tile
from concourse import mybir
from concourse.bass import ds
from concourse.bass_utils import run_bass_kernel_spmd
from concourse.masks import make_identity
import ml_dtypes

# Problem constants (nn_AudioEncoder: 4-layer bidirectional LSTM w/ projection)
B, T, EMBED = 16, 2048, 1024
H = 256      # cell size
P = 128      # proj size
L = 4        # layers
NCORES = 8
B2 = B // NCORES   # sequences per core
NG = 8             # gate chunks of 128 (4H = 1024)
# chunk reorder: pytorch gate order [i,f,g,o] -> [i,f,o,g] so sigmoid gates are contiguous
PERM = [0, 1, 2, 3, 6, 7, 4, 5]

BLK = 32           # recurrent steps per block
UNROLL = 2         # blocks per For_i iteration
NTT = 256          # t-steps per GEMM token tile (512 tokens)

F32 = mybir.dt.float32
BF16 = mybir.dt.bfloat16
AF = mybir.ActivationFunctionType


def build_nc(Tloc=T):
    nt_tiles = Tloc // NTT
    nc = bacc.Bacc()
    # ---- DRAM I/O (per core) ----
    xT = nc.dram_tensor("xT", [NG, 128, Tloc, B2], F32, kind="ExternalInput")
    wih0 = nc.dram_tensor("wih0", [2, 8, 128, 1024], F32, kind="ExternalInput")
    wih123 = nc.dram_tensor("wih123", [3, 2, 2, 128, 1024], BF16, kind="ExternalInput")
    whh = nc.dram_tensor("whh", [L, 2, 128, 1024], BF16, kind="ExternalInput")
    whr = nc.dram_tensor("whr", [L, 2, 2, 128, 128], BF16, kind="ExternalInput")
    bias = nc.dram_tensor("bias", [L, 2, 128, 8], F32, kind="ExternalInput")
    outf = nc.dram_tensor("outf", [128, Tloc, B2], F32, kind="ExternalOutput")
    outb = nc.dram_tensor("outb", [128, Tloc, B2], F32, kind="ExternalOutput")

    with tile.TileContext(nc) as tc:
        with (
            tc.tile_pool(name="dram", bufs=1, space="DRAM") as dpool,
            tc.tile_pool(name="hbufs", bufs=1) as hpool,
        ):
            id_sb = hpool.tile([128, 128], F32, tag="id128", name="id128")
            make_identity(nc, id_sb[:])
            # xp scratch in DRAM: per dir [128, NG, T, B2]
            xp_dram = [dpool.tile([128, NG, Tloc, B2], F32, tag=f"xp{d}", name=f"xp{d}") for d in range(2)]
            # layer-output h buffers (SBUF resident), 2 generations x 2 dirs
            hout = [
                [hpool.tile([128, Tloc, B2], BF16, tag=f"hout{g}{d}", name=f"hout{g}{d}") for d in range(2)]
                for g in range(2)
            ]

            for l in range(L):
                gen, pgen = l % 2, (l - 1) % 2
                kchunks = 8 if l == 0 else 2
                with ExitStack() as layer_ctx:
                    wl = layer_ctx.enter_context(tc.tile_pool(name=f"wl{l}", bufs=1))
                    # ---- load this layer's weights ----
                    wih_sb = wl.tile([128, 2, kchunks, 1024], F32 if l == 0 else BF16, tag="wih")
                    for d in range(2):
                        for k in range(kchunks):
                            src = wih0[d, k] if l == 0 else wih123[l - 1, d, k]
                            nc.gpsimd.dma_start(wih_sb[:, d, k, :], src)
                    whh_sb = wl.tile([128, 2, 1024], BF16, tag="whh")
                    whr_sb = wl.tile([128, 2, 2, 128], BF16, tag="whr")
                    bias_sb = wl.tile([128, 2, 8], F32, tag="bias")
                    for d in range(2):
                        nc.gpsimd.dma_start(whh_sb[:, d, :], whh[l, d])
                        for kc in range(2):
                            nc.gpsimd.dma_start(whr_sb[:, d, kc, :], whr[l, d, kc])
                        nc.gpsimd.dma_start(bias_sb[:, d, :], bias[l, d])

                    # ---- GEMM phase: xp[d] = W_ih @ input + b  (all t) ----
                    with (
                        tc.tile_pool(name=f"gemm{l}", bufs=2) as gp,
                        tc.tile_pool(name=f"gstage{l}", bufs=4) as gsp,
                        tc.tile_pool(name=f"gpsum{l}", bufs=4, space="PSUM") as gps,
                    ):
                        for n in range(nt_tiles):
                            m = nt_tiles - 1 - n  # mirrored tile (reversed-time reads)
                            if l == 0:
                                xtiles = []
                                for k in range(kchunks):
                                    xt = gp.tile([128, NTT, B2], F32, tag=f"xt{k}")
                                    nc.gpsimd.dma_start(
                                        xt[:], xT[k, :, n * NTT:(n + 1) * NTT, :]
                                    )
                                    xtiles.append(xt)
                                rhs_f = xtiles                           # fwd tile n
                                rhs_b = [x[:, ::-1, :] for x in xtiles]  # bwd tile m
                            else:
                                hf, hb = hout[pgen][0], hout[pgen][1]
                                rhs_f = [
                                    hf[:, n * NTT:(n + 1) * NTT, :],
                                    hb[:, m * NTT:(m + 1) * NTT, :][:, ::-1, :],
                                ]
                                rhs_b = [
                                    hf[:, n * NTT:(n + 1) * NTT, :][:, ::-1, :],
                                    hb[:, m * NTT:(m + 1) * NTT, :],
                                ]
                            for d, rhs, tidx in ((0, rhs_f, n), (1, rhs_b, m)):
                                for j in range(NG):
                                    ps = gps.tile([128, NTT, B2], F32, tag="gemmps")
                                    for k in range(kchunks):
                                        nc.tensor.matmul(
                                            ps[:],
                                            wih_sb[:, d, k, j * 128:(j + 1) * 128],
                                            rhs[k],
                                            start=(k == 0),
                                            stop=(k == kchunks - 1),
                                        )
                                    st = gsp.tile([128, NTT, B2], F32, tag="stage")
                                    nc.scalar.activation(
                                        st[:], ps[:], AF.Identity,
                                        bias=bias_sb[:, d, j:j + 1],
                                    )
                                    nc.sync.dma_start(
                                        xp_dram[d][:, j, tidx * NTT:(tidx + 1) * NTT, :],
                                        st[:],
                                    )

                    # ---- recurrent loop phase ----
                    with (
                        tc.tile_pool(name=f"loop{l}", bufs=1) as lp,
                        tc.tile_pool(name=f"tmp{l}", bufs=2) as tp,
                        tc.tile_pool(name=f"lpsum{l}", bufs=1, space="PSUM") as lps,
                    ):
                        c_st = [lp.tile([128, 2, B2], F32, tag=f"c{d}", name=f"c{d}") for d in range(2)]
                        xpb = [
                            [lp.tile([128, NG, BLK, B2], F32, tag=f"xpb{d}{p}", name=f"xpb{d}{p}")
                             for p in range(UNROLL)]
                            for d in range(2)
                        ]
                        hst = [
                            [lp.tile([128, BLK, B2], BF16, tag=f"hst{d}{p}", name=f"hst{d}{p}")
                             for p in range(UNROLL)]
                            for d in range(2)
                        ]
                        ost = [
                            [lp.tile([128, BLK, B2], F32, tag=f"ost{d}{p}", name=f"ost{d}{p}")
                             for p in range(UNROLL)]
                            for d in range(2)
                        ] if l == L - 1 else None
                        psg = [lps.tile([128, NG, B2], F32, tag=f"psg{d}", name=f"psg{d}") for d in range(2)]
                        psh = [lps.tile([128, B2], F32, tag=f"psh{d}", name=f"psh{d}") for d in range(2)]

                        for d in range(2):
                            nc.vector.memset(c_st[d][:], 0.0)
                            nc.vector.memset(hst[d][UNROLL - 1][:, BLK - 1, :], 0.0)

                        with tc.For_i(
                            0, Tloc, UNROLL * BLK,
                            staggered_reset=True,
                            hint_engines=(
                                mybir.EngineType.PE, mybir.EngineType.Activation,
                                mybir.EngineType.DVE, mybir.EngineType.SP,
                            ),
                        ) as i0:
                            for pp in range(UNROLL):
                                off = i0 + pp * BLK
                                for d in range(2):
                                    nc.sync.dma_start(
                                        xpb[d][pp][:], xp_dram[d][:, :, ds(off, BLK), :]
                                    )
                                for k in range(BLK):
                                    for d in range(2):
                                        h_prev = (
                                            hst[d][pp][:, k - 1, :] if k > 0
                                            else hst[d][pp - 1][:, BLK - 1, :]
                                        )
                                        gv = psg[d]
                                        nc.tensor.matmul(
                                            gv[:], id_sb[:], xpb[d][pp][:, :, k, :],
                                            start=True, stop=False, skip_group_check=True,
                                        )
                                        for j in range(NG):
                                            nc.tensor.matmul(
                                                gv[:, j, :],
                                                whh_sb[:, d, j * 128:(j + 1) * 128],
                                                h_prev,
                                                start=False, stop=(j == NG - 1),
                                                skip_group_check=True,
                                            )
                                        sg = tp.tile([128, NG, B2], F32, tag=f"sg{d}")
                                        nc.scalar.activation(sg[:], gv[:], AF.Sigmoid)
                                        gt = tp.tile([128, 2, B2], F32, tag=f"gt{d}")
                                        nc.vector.tensor_scalar(
                                            gt[:], sg[:, 6:8, :], 2.0, -1.0,
                                            mybir.AluOpType.mult, mybir.AluOpType.add,
                                        )
                                        ig = tp.tile([128, 2, B2], F32, tag=f"ig{d}")
                                        nc.vector.tensor_mul(ig[:], sg[:, 0:2, :], gt[:])
                                        fc = tp.tile([128, 2, B2], F32, tag=f"fc{d}")
                                        nc.vector.tensor_mul(fc[:], sg[:, 2:4, :], c_st[d][:])
                                        nc.vector.tensor_add(c_st[d][:], ig[:], fc[:])
                                        s2c = tp.tile([128, 2, B2], F32, tag=f"s2c{d}")
                                        nc.scalar.activation(s2c[:], c_st[d][:], AF.Sigmoid,
                                                             scale=2.0)
                                        t1 = tp.tile([128, 2, B2], F32, tag=f"t1{d}")
                                        nc.vector.tensor_mul(t1[:], sg[:, 4:6, :], s2c[:])
                                        s_t = tp.tile([128, 2, B2], BF16, tag=f"s{d}")
                                        nc.vector.scalar_tensor_tensor(
                                            s_t[:], t1[:], 2.0, sg[:, 4:6, :],
                                            mybir.AluOpType.mult, mybir.AluOpType.subtract,
                                        )
                                        nc.tensor.matmul(
                                            psh[d][:], whr_sb[:, d, 0, :], s_t[:, 0, :],
                                            start=True, stop=False,
                                        )
                                        nc.tensor.matmul(
                                            psh[d][:], whr_sb[:, d, 1, :], s_t[:, 1, :],
                                            start=False, stop=True,
                                        )
                                        nc.vector.tensor_copy(hst[d][pp][:, k, :], psh[d][:])
                                        if l == L - 1:
                                            nc.scalar.copy(ost[d][pp][:, k, :], psh[d][:])
                                # flush h block
                                for d in range(2):
                                    if l < L - 1:
                                        nc.sync.dma_start(
                                            hout[gen][d][:, ds(off, BLK), :], hst[d][pp][:]
                                        )
                                    else:
                                        nc.sync.dma_start(
                                            (outf if d == 0 else outb)[:, ds(off, BLK), :],
                                            ost[d][pp][:],
                                        )
    nc.compile()
    return nc


def _prep_inputs(x, params, Tloc=T):
    """Host-side: shard + layout. Returns list of per-core input dicts."""
    perm = np.array(PERM)
    row_perm = (perm[:, None] * 128 + np.arange(128)[None, :]).reshape(-1)
    wih0_a = np.zeros((2, 8, 128, 1024), np.float32)
    wih123_a = np.zeros((3, 2, 2, 128, 1024), ml_dtypes.bfloat16)
    whh_a = np.zeros((L, 2, 128, 1024), ml_dtypes.bfloat16)
    whr_a = np.zeros((L, 2, 2, 128, 128), ml_dtypes.bfloat16)
    bias_a = np.zeros((L, 2, 128, 8), np.float32)
    for l in range(L):
        for d in range(2):
            W_ih, W_hh, b_ih, b_hh, W_hr = [np.asarray(a, np.float32) for a in params[l][d]]
            W_ih = W_ih[row_perm].copy()       # [1024, in_dim]
            W_hh = W_hh[row_perm].copy()       # [1024, 128]
            bb = (b_ih + b_hh)[row_perm].copy()
            W_ih[768:] *= 2.0; W_hh[768:] *= 2.0; bb[768:] *= 2.0
            if l == 0:
                wih0_a[d] = np.ascontiguousarray(W_ih.T.reshape(8, 128, 1024))
            else:
                wih123_a[l - 1, d] = np.ascontiguousarray(W_ih.T.reshape(2, 128, 1024)).astype(ml_dtypes.bfloat16)
            whh_a[l, d] = W_hh.T.astype(ml_dtypes.bfloat16)
            whr_a[l, d] = W_hr.T.reshape(2, 128, 128).astype(ml_dtypes.bfloat16)
            bias_a[l, d] = bb.reshape(8, 128).T
    ins = []
    for c in range(NCORES):
        xc = np.asarray(x[c * B2:(c + 1) * B2, :Tloc], np.float32)  # [B2, T, E]
        xTc = np.ascontiguousarray(xc.transpose(2, 1, 0)).reshape(8, 128, Tloc, B2)
        ins.append(dict(xT=xTc, wih0=wih0_a, wih123=wih123_a, whh=whh_a,
                        whr=whr_a, bias=bias_a))
    return ins


_NC_CACHE = {}


def kernel(x, params):
    x = np.asarray(x)
    Tloc = x.shape[1]
    if Tloc not in _NC_CACHE:
        _NC_CACHE[Tloc] = build_nc(Tloc)
    nc = _NC_CACHE[Tloc]
    ins = _prep_inputs(x, params, Tloc)
    res = run_bass_kernel_spmd(nc, ins, list(range(NCORES))).results
    out = np.zeros((B, Tloc, 2 * P), np.float32)
    for c in range(NCORES):
        hf = res[c]["outf"]  # [128, T, B2] natural t
        hb = res[c]["outb"]  # [128, T, B2] tau = T-1-t
        out[c * B2:(c + 1) * B2, :, :P] = hf.transpose(2, 1, 0)
        out[c * B2:(c + 1) * B2, :, P:] = hb[:, ::-1, :].transpose(2, 1, 0)
    return out
